# revision 35
# baseline (speedup 1.0000x reference)
"""GAT-style attention kernel for Trainium2, data-parallel over batch on 8 cores.

Math: the reference computes
    e[i,j]  = lr_row[i] + lr_col[j]            (rank-1 score structure)
    atten   = softmax_j(where(mask>0, e, -1e9))
    out     = atten @ (x @ Wx.T + bx)
lr_row[i] is constant along the softmax axis j, so it cancels:
    atten[i,j] = mask[i,j] * w[j] / sum_j mask[i,j] * w[j],  w[j] = exp(lr_col[j])
(no max-subtraction needed: lr_col in [-0.4, 1.6] for this distribution)
and since attention rows sum to 1, the bias folds into the numerator:
    out = (M @ (w * (xv0 + bx))) / (M @ w),   xv0 = x @ Wx.T
So the whole kernel is one [N,N] x [N,130] matmul per batch, normalized
row-wise, with tiny setup.  Memory-bound on the mask read.

Implementation notes (hard-won on HW):
  - Mask is host pre-tiled to fp8 (0/1 exact); U = w*(xv+bx) is built in fp8
    e4m3 so the main loop runs DoubleRow (2 j-tiles per matmul, both
    operands fp8, ~78ns per K=256 pair vs 2x58ns for bf16).  Total rel err
    ~9.9e-3 (vs 1.3e-3 bf16), well under the 2e-2 gate; verified in numpy
    and bit-faithful on HW.
  - Mask streams as SEVEN transfers (c0..c3, c45, c6, c7) on the sync ring
    behind the consts: fine-grained where the loop rides the arrival front,
    coarse mid-stream.  Keeping 3+ transfers in flight matters (single
    transfers only reach ~200GB/s; the wire peaks ~420GB/s with several).
  - Engine assignment is the core of the schedule.  PE: warmups, col MMs,
    K=1 matmuls that PRE-FILL the xv PSUM banks with bx (so no separate
    bias add exists anywhere), xv MMs accumulating on top, then the
    DoubleRow main loop.  ACT: the score chain as
    w = exp(a2_0*lrelu(col_0)) * exp(a2_1*lrelu(col_1)) via Lrelu/Exp
    activations reading col straight from PSUM (per-partition scale APs).
    GPSIMD: the tiny w products and fp8 denominator-column copies
    (SBUF-only ops).  DVE: ONLY the four [P,4,128] broadcast multiplies
    that build U (stride-0 broadcast of w over the free dim) and the strip
    tails.  This keeps the U critical path off any busy queue.
  - PSUM zero regions are 2KB: one OPEN accumulation group per bank.  The
    first 4 strips accumulate in 4 separate banks (their groups stay open
    across the U-half boundary); later strips run whole-strip sequential
    in 2KB pair tiles (sequential groups may share a bank).
  - Strip tails: one reciprocal over the packed denominators + one
    broadcast multiply straight out of PSUM into bf16 output pairs (bias
    already folded into U).  bf16 stores halve store traffic (~3e-4 err);
    host upcasts.
"""

import os
import sys

import numpy as np

for _p in ("/opt/trn_rl_repo",):
    if _p not in sys.path and os.path.isdir(_p):
        sys.path.append(_p)

import concourse.bacc as bacc
import concourse.bass as bass
import concourse.bass_isa as bass_isa
import concourse.tile as tile
from concourse import mybir
from concourse.bass_utils import run_bass_kernel_spmd

B, N, DIN, DOUT, DA = 8, 2048, 128, 128, 2
NEG_SLOPE = 0.2
P = 128
NT = N // P
UC = 130  # U free width: 128 numerator cols + 1 denom col + 1 pad
CW = DOUT + DA  # proj width

F32 = mybir.dt.float32
BF16 = mybir.dt.bfloat16
FP8 = mybir.dt.float8e4

N_CHUNKS = 8
N_WARM1 = 10  # dummy PE warm-up matmuls before proj
NH = NT // 2  # x^T tiles per half-chain
NA1 = 4  # x^T tiles in cbfA1a (rest of h0 in cbfA1b)
# mask transfer grouping: (first chunk, chunk count) per DMA
M_SPECS = ((0, 1), (1, 1), (2, 1), (3, 1), (4, 2), (6, 1), (7, 1))


def build(n_chunks=N_CHUNKS):
    """Build the single-core program (all 8 cores run it SPMD)."""
    nt = NT
    spc = nt // n_chunks  # strips per chunk
    nc = bacc.Bacc(
        "TRN2",
        target_bir_lowering=False,
        debug=False,
        enable_asserts=False,
        num_devices=1,
    )
    # maskt[c, jj, s, tj, ii] = mask[(c*spc+s)*128+ii, tj*128+jj]  (host-tiled)
    m_d = nc.dram_tensor(
        "maskt", [n_chunks, P, spc, nt, P], FP8, kind="ExternalInput"
    ).ap()
    cbfA1_d = nc.dram_tensor("cbfA1", [P, CW + NH * P], BF16, kind="ExternalInput").ap()
    cbfA2_d = nc.dram_tensor("cbfA2", [P, NH * P], BF16, kind="ExternalInput").ap()
    cf32_d = nc.dram_tensor("cf32", [P, DA + DOUT], F32, kind="ExternalInput").ap()
    cbx_d = nc.dram_tensor("cbx", [1, 4 * P], BF16, kind="ExternalInput").ap()
    # output in store-friendly layout: [pair, p, t, o] with contiguous 512B
    # rows per partition; host reassembles to [N, DOUT] (pure reshape)
    out_d = nc.dram_tensor(
        "out", [NT // 2, P, 2, DOUT], BF16, kind="ExternalOutput"
    ).ap()

    from contextlib import ExitStack

    with tile.TileContext(nc) as tc, ExitStack() as ctx:
        consts = ctx.enter_context(tc.tile_pool(name="consts", bufs=1))
        small = ctx.enter_context(tc.tile_pool(name="small", bufs=1))
        mpool = ctx.enter_context(tc.tile_pool(name="mpool", bufs=1))
        opool = ctx.enter_context(tc.tile_pool(name="opool", bufs=4))
        ps_proj = ctx.enter_context(tc.tile_pool(name="ps_proj", bufs=1, space="PSUM"))
        ps_acc = ctx.enter_context(tc.tile_pool(name="ps_acc", bufs=4, space="PSUM"))

        # ---- input DMAs.  The wire round-robins ACTIVE transfers with no
        # priority, and a single transfer only reaches ~100-200GB/s, so the
        # consts go as FOUR parallel streams (aggregate ~400GB/s) and the
        # first mask DMAs are GATED behind them (GPS reads each const's
        # tail, then a memset on the mask tile gives the DMA a WAW dep) ----
        cbfA1 = consts.tile([P, CW + NA1 * P], BF16)
        nc.sync.dma_start(cbfA1[:], cbfA1_d[:, 0 : CW + NA1 * P])
        cbfA2 = consts.tile([P, NH * P], BF16)
        nc.sync.dma_start(cbfA2[:], cbfA2_d)
        cbfA1b = consts.tile([P, (NH - NA1) * P], BF16)
        nc.sync.dma_start(cbfA1b[:], cbfA1_d[:, CW + NA1 * P :])
        cbx = consts.tile([1, 4 * P], BF16)
        nc.scalar.dma_start(cbx[:], cbx_d)
        cf32 = consts.tile([P, DA + DOUT], F32)
        nc.scalar.dma_start(cf32[:], cf32_d)
        wcomb = cbfA1[:, 0:CW]
        a2b = cf32[:, 0:DA]

        def xt_chunk(t):
            if t < NA1:
                return cbfA1[:, CW + t * P : CW + (t + 1) * P]
            if t < NH:
                return cbfA1b[:, (t - NA1) * P : (t - NA1 + 1) * P]
            return cbfA2[:, (t - NH) * P : (t - NH + 1) * P]

        # mask transfers behind the consts on sync; only m0 is gated on
        # the consts' completion (GPS reads each const tail, then a memset
        # arms m0's WAW dep) -- the issue stagger orders the rest
        gatev = consts.tile([1, 8], F32)
        for gi, gsrc in enumerate(
            (cbfA1[0:1, CW + NA1 * P - 1 :], cbfA2[0:1, NH * P - 1 :],
             cbfA1b[0:1, (NH - NA1) * P - 1 :])
        ):
            nc.gpsimd.tensor_copy(gatev[:, gi : gi + 1], gsrc)
        mtiles = []
        for mi, (c0_, w_) in enumerate(M_SPECS):
            if w_ == 1:
                t_ = mpool.tile([P, spc, nt, P], FP8, name=f"m{c0_}")
                if mi < 1:
                    nc.gpsimd.memset(t_[0:1, 0, 0, 0:1], 0)
                nc.sync.dma_start(t_[:], m_d[c0_])
            else:
                t_ = mpool.tile([P, w_, spc, nt, P], FP8, name=f"m{c0_}")
                nc.sync.dma_start(
                    t_[:], m_d[c0_ : c0_ + w_].rearrange("c p s t i -> p c s t i")
                )
            mtiles.append((c0_, w_, t_))

        def mpair(ti, tp):
            c, s = ti // spc, ti % spc
            for c0_, w_, t_ in mtiles:
                if c0_ <= c < c0_ + w_:
                    if w_ == 1:
                        return t_[:, s, 2 * tp : 2 * tp + 2]
                    return t_[:, c - c0_, s, 2 * tp : 2 * tp + 2]
            raise AssertionError(ti)

        # ---- PE warm-up: bridge the preamble idle window so the HAM clock
        # gate ramps before the projection matmuls ----
        wa = consts.tile([P, P], FP8)
        nc.vector.memset(wa[:], 0)
        wb = consts.tile([P, UC], BF16)
        nc.vector.memset(wb[:], 0)
        ones1 = consts.tile([1, P], FP8)
        nc.vector.memset(ones1[:], 1.0)
        for _ in range(N_WARM1):
            pw = ps_acc.tile([P, 2, 256], F32, tag="acc")
            nc.tensor.matmul(pw[:, 0, 0:UC], wa[:], wb[:], start=True, stop=True)

        # U pad col cleared early (no deps); fp8 for the DoubleRow main loop
        U = consts.tile([P, nt, UC], FP8)
        nc.vector.memset(U[:, :, DOUT + 1 : UC], 0)
        w_all = consts.tile([P, nt], F32)

        pcols = {}
        pxv8s = {}

        def col_pass(h):
            t0 = h * NH
            pcol = ps_acc.tile([P, NH, DA], F32, tag="acc", name=f"pcol{h}")
            for i in range(NH):
                nc.tensor.matmul(
                    pcol[:, i], xt_chunk(t0 + i), wcomb[:, DOUT : DOUT + DA],
                    start=True, stop=True,
                )
            pcols[h] = pcol

        def bx_prefill(h):
            # K=1 matmuls write bx into both xv PSUM banks; xv MMs accumulate
            pxv8 = ps_proj.tile([P, NH, DOUT], F32, tag=f"pxv8_{h}", name=f"pxv8_{h}")
            for q in range(2):
                nc.tensor.matmul(
                    pxv8[:, 4 * q : 4 * q + 4], ones1[:], cbx[:],
                    start=True, stop=False, skip_group_check=True,
                )
            pxv8s[h] = pxv8

        def xv_pass(h):
            t0 = h * NH
            pxv8 = pxv8s[h]
            for i in range(NH):
                nc.tensor.matmul(
                    pxv8[:, i], xt_chunk(t0 + i), wcomb[:, 0:DOUT],
                    start=False, stop=(i % 4 == 3), skip_group_check=True,
                )

        def score_chain(h):
            # lrelu via max-STT, fold a2 with a stride-0 broadcast multiply,
            # sum the DA=2 slices, ONE exp on ACT: w = exp(sum_a a2_a*lr_a).
            # h0 runs on DVE (fastest path to the first U tiles); h1's
            # elementwise ops go to ACT(copy)+GPS so they never sit in the
            # DVE FIFO ahead of h0's U multiplies.  den copies on GPS.
            t0 = h * NH
            pcol = pcols[h]
            colv = small.tile([P, NH, DA], F32, name=f"colv{h}")
            if h == 0:
                nc.vector.tensor_copy(colv[:], pcol[:])
            else:
                nc.scalar.copy(colv[:], pcol[:])
            clr = small.tile([P, NH, DA], F32, name=f"clr{h}")
            nc.vector.scalar_tensor_tensor(
                clr[:], colv[:], NEG_SLOPE, colv[:],
                mybir.AluOpType.mult, mybir.AluOpType.max,
            )
            ca = small.tile([P, NH, DA], F32, name=f"ca{h}")
            nc.vector.tensor_tensor(
                ca[:], clr[:],
                a2b.unsqueeze(1).to_broadcast([P, NH, DA]),
                mybir.AluOpType.mult,
            )
            lrc = small.tile([P, NH], F32, name=f"lrc{h}")
            nc.vector.tensor_tensor(
                lrc[:], ca[:, :, 0], ca[:, :, 1], mybir.AluOpType.add
            )
            nc.scalar.activation(
                w_all[:, t0 : t0 + NH], lrc[:], mybir.ActivationFunctionType.Exp
            )
            # denominator column in fp8 (error averages out over the row sum)
            nc.gpsimd.tensor_copy(U[:, t0 : t0 + NH, DOUT], w_all[:, t0 : t0 + NH])

        def u_mult_pair(tp):
            # one [P,2,128] broadcast multiply on DVE builds U for DR pair tp
            t0 = 2 * tp
            h = t0 // NH
            pxv8 = pxv8s[h]
            o = t0 - h * NH
            nc.vector.tensor_tensor(
                U[:, t0 : t0 + 2, 0:DOUT], pxv8[:, o : o + 2],
                w_all[:, t0 : t0 + 2].unsqueeze(2).to_broadcast([P, 2, DOUT]),
                mybir.AluOpType.mult,
            )

        # ---- main loop pieces: DoubleRow fp8, 2 j-tiles per matmul ----
        def strip_mms(ti, pacc, tps):
            for tp in tps:
                nc.tensor.matmul(
                    pacc[:],
                    mpair(ti, tp),
                    U[:, 2 * tp : 2 * tp + 2, :],
                    start=(tp == 0),
                    stop=(tp == nt // 2 - 1),
                    perf_mode=mybir.MatmulPerfMode.DoubleRow,
                )

        opairs = {}

        def store_pair(pi, o2):
            dst = out_d[pi]
            if pi == nt // 2 - 1:
                hp = P // 2
                nc.sync.dma_start(dst[0:hp], o2[0:hp])
                nc.scalar.dma_start(dst[hp:P], o2[hp:P])
            else:
                eng = nc.scalar if pi % 2 == 0 else nc.sync
                eng.dma_start(dst, o2[:])

        def ilv_tail(ti, pacc):
            rec = small.tile([P, 1], F32, name=f"reci{ti}")
            nc.vector.reciprocal(rec[:], pacc[:, DOUT : DOUT + 1])
            pi, h = ti // 2, ti % 2
            if h == 0:
                opairs[pi] = opool.tile([P, 2, DOUT], BF16, tag="o2", name=f"o2_{pi}")
            o2 = opairs[pi]
            nc.vector.tensor_scalar(
                o2[:, h], pacc[:, 0:DOUT], rec[:], None, mybir.AluOpType.mult
            )
            if h == 1:
                store_pair(pi, o2)

        def pair_tail(pi, pacc2):
            rec2 = small.tile([P, 2], F32, name=f"rec{pi}")
            nc.vector.reciprocal(rec2[:], pacc2[:, :, DOUT])
            o2 = opool.tile([P, 2, DOUT], BF16, tag="o2", name=f"o2_{pi}")
            nc.vector.tensor_tensor(
                o2[:], pacc2[:, :, 0:DOUT],
                rec2.unsqueeze(2).to_broadcast([P, 2, DOUT]),
                mybir.AluOpType.mult,
            )
            store_pair(pi, o2)

        # ---- schedule (emission order = per-engine FIFO order) ----
        bx_prefill(0)        # PE 2x K=1 N=512 MMs during cbfA1's flight
        col_pass(0)          # PE 8x F=2 MMs, gated on cbfA1
        col_pass(1)          # gated on cbfA2
        score_chain(0)       # DVE chain + ACT exp
        xv_pass(0)           # gated on cbfA1/b
        for _tp in range(4):
            u_mult_pair(_tp)     # DVE; U pairs 0..3 (h0)
        score_chain(1)       # h1 chain on DVE behind h0's U pairs
        bx_prefill(1)
        xv_pass(1)
        for _tp in range(4, 8):
            u_mult_pair(_tp)     # DVE; U pairs 4..7 (h1)

        # first 4 strips: 4 separate single-bank accumulators whose groups
        # stay open across the U-half boundary (one open group per 2KB
        # zero region), sequential per strip within each half
        ilv_paccs = [
            ps_acc.tile([P, 2, 256], F32, tag="acc", name=f"ilvp{i}")
            for i in range(4)
        ]
        for ti in range(4):
            strip_mms(ti, ilv_paccs[ti][:, 0, 0:UC], range(nt // 4))
        for ti in range(4):
            strip_mms(ti, ilv_paccs[ti][:, 0, 0:UC], range(nt // 4, nt // 2))
        for ti in range(4):
            ilv_tail(ti, ilv_paccs[ti][:, 0, 0:UC])
        for pi in range(2, nt // 2 - 1):
            pacc2 = ps_acc.tile([P, 2, 256], F32, tag="acc")
            strip_mms(2 * pi, pacc2[:, 0, 0:UC], range(nt // 2))
            strip_mms(2 * pi + 1, pacc2[:, 1, 0:UC], range(nt // 2))
            pair_tail(pi, pacc2[:, :, 0:UC])
        # last pair: per-strip tails so strip 14's normalize/store overlaps
        # strip 15's matmuls, shortening the final drain
        pacc2 = ps_acc.tile([P, 2, 256], F32, tag="acc")
        strip_mms(nt - 2, pacc2[:, 0, 0:UC], range(nt // 2))
        ilv_tail(nt - 2, pacc2[:, 0, 0:UC])
        strip_mms(nt - 1, pacc2[:, 1, 0:UC], range(nt // 2))
        ilv_tail(nt - 1, pacc2[:, 1, 0:UC])

    nc.compile()
    return nc


def host_inputs(x, mask, Wc, Wcat, Wx, bx, b, n_chunks=N_CHUNKS):
    """Per-core input map for batch b: layout/dtype prep only (no math)."""
    import ml_dtypes

    bf16 = ml_dtypes.bfloat16
    fp8 = ml_dtypes.float8_e4m3fn
    spc = NT // n_chunks
    # maskt[c, jj, s, tj, ii] = mask[b][(c*spc+s)*128+ii, tj*128+jj]
    mt = np.ascontiguousarray(
        np.asarray(mask[b])
        .reshape(n_chunks, spc, P, NT, P)
        .transpose(0, 4, 1, 3, 2)
        .astype(fp8)
    )
    wc = np.concatenate([Wx.T, Wc.T], axis=1)
    xTb = np.asarray(x[b]).T
    cbfA1 = np.concatenate([wc, xTb[:, : NH * P]], axis=1).astype(bf16)
    cbfA2 = xTb[:, NH * P :].astype(bf16)
    cf32 = np.concatenate(
        [
            np.broadcast_to(Wcat[DA:].reshape(1, DA), (P, DA)),
            np.broadcast_to(bx.reshape(1, DOUT), (P, DOUT)),
        ],
        axis=1,
    ).astype(np.float32)
    cbx = np.tile(np.asarray(bx), 4).reshape(1, 4 * P).astype(bf16)
    return {
        "maskt": mt,
        "cbfA1": np.ascontiguousarray(cbfA1),
        "cbfA2": np.ascontiguousarray(cbfA2),
        "cf32": np.ascontiguousarray(cf32),
        "cbx": np.ascontiguousarray(cbx),
    }


_cached = {}


def _get_nc(n_chunks=N_CHUNKS):
    if n_chunks not in _cached:
        _cached[n_chunks] = build(n_chunks)
    return _cached[n_chunks]


def _install_ntff_shim():
    """The agent image's antenv lacks axon_hooks; synthesize it so
    run_bass_kernel_spmd(trace=True) can reach the .so's NTFF profiler."""
    import types

    try:
        import antenv.axon_hooks  # noqa: F401

        return True
    except ImportError:
        pass
    try:
        import antenv
        from trn_agent_boot.trn_boot import _ntff_profile_via_ctypes

        hook = _ntff_profile_via_ctypes("/opt/axon/libaxon_pjrt.so")
        mod = types.ModuleType("antenv.axon_hooks")
        _state = {"hook": hook}
        mod.set_axon_ntff_profile_hook = lambda h: _state.__setitem__("hook", h)
        mod.get_axon_ntff_profile_hook = lambda: _state["hook"]
        sys.modules["antenv.axon_hooks"] = mod
        antenv.axon_hooks = mod
        return hook is not None
    except Exception as e:
        print(f"ntff shim failed: {e}", file=sys.stderr)
        return False


def kernel(x, mask, Wr, Wc, Wcat, Wx, bx, _trace=False,
           _n_chunks=N_CHUNKS, **_unused):
    x = np.asarray(x)
    mask = np.asarray(mask)
    Wc = np.asarray(Wc)
    Wcat = np.asarray(Wcat)
    Wx = np.asarray(Wx)
    bx = np.asarray(bx)
    nc = _get_nc(_n_chunks)
    if _trace:
        _trace = _install_ntff_shim()
    in_maps = [
        host_inputs(x, mask, Wc, Wcat, Wx, bx, b, _n_chunks) for b in range(B)
    ]
    res = run_bass_kernel_spmd(nc, in_maps, core_ids=list(range(B)), trace=_trace)
    # out comes back as [NT//2, P, 2, DOUT]; reassemble rows (pi, t, p)
    out = np.stack(
        [
            np.asarray(res.results[c]["out"])
            .transpose(0, 2, 1, 3)
            .reshape(N, DOUT)
            for c in range(B)
        ]
    ).astype(np.float32)
    if _trace:
        kernel.last_results = res
    return out
